# revision 15
# baseline (speedup 1.0000x reference)
"""FRQI encoding kernel for Trainium2 (8 NeuronCores, data-parallel).

Closed form of the reference: for each sample b with 4 pixels x[b, 0:4],
  out[b] = [0.0, 0.0, mean_i cos(x[b, i] * pi / 255)]
The two address-qubit columns are input-independent and exactly zero
(mean over 4 pixel indices of (-1)^bit is 0 for both address bits), so
they are filled on the host; the device computes only the color column.

The ScalarE Sin stream (1 elem/cycle/lane @ 1.2 GHz -> ~15 us/core for
the 2.1M pixels) is the hard floor, so every other resource is sized to
never stall it, staying well inside the 2e-2 rel-err gate:
  - inputs are quantized to uint8 on the host (np.rint; the data is
    8-bit pixel intensities, quantization adds ~3.6e-3 rel err):
    2 MiB/core read. fp16 inputs (4 MiB) measurably starved the ACT
    stream in the early phase (effective supply ~4.5 us/MiB with
    dispatch+receipt overheads vs ACT demand ~3.6 us/MiB-equivalent).
  - the device emits the per-sample SUM of the 4 cos values in fp16
    (1 MiB/core); the host applies the 1/4 and writes the zero columns.

Device kernel (per core, 524288 samples = 2097152 uint8 pixels):
  - tiles of (128 partitions x F u8), contiguous DMA in; ladder
    schedule: tile F grows at the rate the measured DMA supply curve
    (sem_k ~ 8.9us + 0.55ns/elem) stays ahead of ACT consumption
    (0.833 ns/elem), so the Sin stream runs gap-free from first tile;
    small last tiles shrink the trailing adds+store.
  - ScalarE activation Sin(pi/2 - x*pi/255) == +cos(2*theta), u8 in /
    fp16 out into two alternating buffers (the HW Sin spline is only
    accurate on ~[-pi, pi]; the +pi/2 bias keeps arguments in
    (-pi/2, pi/2]).
  - a dependency-free 1-element Sin is issued right after the tile-0
    DMA dispatch so insert_act_table_loads places the Sin-set
    ACT_TABLE_LOAD (~1.3 us) before it, overlapping tile 0's DMA
    instead of serializing after its semaphore.
  - the host pre-groups each tile row as [pix0-blk|pix1-blk|pix2-blk|
    pix3-blk] so the sum-of-4 is three CONTIGUOUS fp16 tensor_adds on
    VectorE: step-1 16-bit operands hit the 2x_1P packed mode (stride-2
    pairwise adds would fall back to 1x and ~double DVE time).
  - contiguous fp16 DMA out of the (128, F/4) sum tile; all stores
    sit behind all loads in the Sync ring's FIFO so they never steal
    SDMA bandwidth from loads that gate compute.
"""

import math
import sys

for _p in ("/opt/trn_rl_repo",):
    if _p not in sys.path:
        sys.path.append(_p)

import numpy as np

# If the environment forces tracing (BASS_TRACE=1), run_bass_kernel_spmd
# imports antenv.axon_hooks, which this image lacks — stub it (only when
# absent) so the trace path degrades to "hook isn't registered" instead
# of crashing the kernel.
try:
    import antenv.axon_hooks  # noqa: F401
except ImportError:
    import types as _types

    _m = _types.ModuleType("antenv.axon_hooks")
    _m.get_axon_ntff_profile_hook = lambda: None
    _m.set_axon_ntff_profile_hook = lambda h: None
    sys.modules["antenv.axon_hooks"] = _m

import concourse.bass as bass
import concourse.mybir as mybir
from concourse import bacc
from concourse.bass_utils import run_bass_kernel_spmd
from concourse.tile import TileContext

N_CORES = 8
B = 4_194_304
N_PIX = 4
N_PER_CORE = B // N_CORES          # 524288 samples
P = 128                            # SBUF partitions
L = N_PER_CORE * N_PIX             # 2097152 u8 pixels per core
LO = N_PER_CORE                    # 524288 fp16 sums per core

# Per-tile free-dim sizes (u8 elems per partition; sum = L/P = 16384).
# Big tiles sit mid-stream: a late 4096-tile's three DVE adds (~1.9 us)
# would outlive the short trailing sins and push the last store out.
# Early tiles stay <=1024: completion semaphores arrive at a ~2 us
# cadence under load regardless of size, so fine granularity keeps the
# Sin stream fed during the supply ramp.
F_SCHED = [512, 1024, 2048, 4096, 4096, 1536, 1536, 1024, 512]
# Tile 5 is computed by a degree-5 odd polynomial on the (otherwise
# half-idle) VectorE instead of ScalarE Sin: cos(pi*x/255) =
# sin(pi/2*u) for u = 1 - 2x/255, evaluated as ((c5*s + c3)*s + c1)*u
# with s = u^2 (max err 1.1e-4, far below the u8 quantization noise).
# This shaves ~1.3 us off the critical ScalarE stream; the poly's DVE
# ops slot into the measured mid-stream DVE idle gaps. Its load is a
# GpSimd/SWDGE DMA that casts u8->fp16 in flight (HWDGE can't cast),
# so the cast costs no engine time.
POLY_T = 5
assert sum(F_SCHED) * P == L
C_SCHED = [f // N_PIX for f in F_SCHED]
F_MAX = max(F_SCHED)
_C1, _C3, _C5 = 1.5706268, -0.6432292, 0.0727102

# cos(z) = sin(pi/2 - z) for z = x*pi/255 = 2*theta: with scale=-pi/255
# and bias=+pi/2 the activation argument stays in (-pi/2, pi/2], the
# accurate domain of the HW Sin spline (it degrades badly beyond ~pi),
# and no sign fix-up is needed downstream.
_SCALE = -math.pi / 255.0
_BIAS = math.pi / 2.0


def _make_bacc() -> bacc.Bacc:
    """Construct Bacc without its init-time const-AP memsets and
    all-engine barrier. Nothing reads the four built-in const APs here
    (birverifier reports all four as "no reader"), and without the
    barrier each engine reaches its first kernel instruction as soon as
    its own runtime prolog finishes — the Activation engine then
    dispatches the first input DMA ~2 us before the Sync engine could.
    The patched methods are restored before any kernel instruction is
    traced."""
    sh = bass.BassSharedVectorInterface
    saved_memset = sh.memset
    saved_barrier = bass.Bass.all_engine_barrier
    sh.memset = lambda self, ap, constant: None
    bass.Bass.all_engine_barrier = lambda self, *a, **k: None
    try:
        return bacc.Bacc()
    finally:
        sh.memset = saved_memset
        bass.Bass.all_engine_barrier = saved_barrier


def _build_nc() -> bass.Bass:
    # Bacc (not raw Bass): its compile() pass generate_event_semaphores
    # splits multi-sem waits to satisfy the 1-wait-per-instruction HW limit.
    nc = _make_bacc()
    u8 = mybir.dt.uint8
    f16 = mybir.dt.float16
    f32 = mybir.dt.float32
    x = nc.dram_tensor("x", [L], u8, kind="ExternalInput")
    y = nc.dram_tensor("y", [LO], f16, kind="ExternalOutput")

    bias_t = nc.alloc_sbuf_tensor("bias_pi2", [P, 1], f32)
    scratch = nc.alloc_sbuf_tensor("act_warm", [P, 1], f16)
    # Persistent per-tile output buffers: stores never WAR with later
    # tiles' compute.
    obufs = [
        nc.alloc_sbuf_tensor(f"ob{t}", [P, c], f16)
        for t, c in enumerate(C_SCHED)
    ]
    bias_ap = bias_t.ap()

    with TileContext(nc) as tc:
        # One slot per uniquely-tagged input tile: no slot reuse, so no
        # in-DMA ever carries a WAR wait and the Sync sequencer can
        # dispatch every input DMA up front.
        with tc.tile_pool(name="io", bufs=1) as pool:
            nc.gpsimd.memset(bias_ap, _BIAS)
            in_off = 0
            out_off = 0
            stores = []
            for t, F in enumerate(F_SCHED):
                C = F // N_PIX
                x_t = x[in_off:in_off + P * F].rearrange("(p f) -> p f", p=P)
                y_t = y[out_off:out_off + P * C].rearrange(
                    "(p f) -> p f", p=P
                )
                if t == POLY_T:
                    # Normal HWDGE u8 load; the u8->fp16 conversion is
                    # folded into the first tensor_scalar (engines read
                    # any dtype and compute in fp32). A SWDGE
                    # cast-during-DMA load was tried and was ~1.3 us
                    # cheaper on DVE, but produced intermittent
                    # corruption (nondeterministic partial-tile reads).
                    pf = pool.tile([P, F], u8, tag="pf")
                    nc.sync.dma_start(out=pf[:], in_=x_t)
                    pu = pool.tile([P, F], f16, tag="pu")
                    ps = pool.tile([P, F], f16, tag="ps")
                    pw = pool.tile([P, F], f16, tag="pw")
                    pw2 = pool.tile([P, F], f16, tag="pw2")
                    pw3 = pool.tile([P, F], f16, tag="pw3")
                    st = pool.tile([P, F_MAX], f16, tag="py")
                    mul = mybir.AluOpType.mult
                    add = mybir.AluOpType.add
                    with nc.allow_low_precision(
                        "fp16 poly cosine; output gate is 2e-2 rel err"
                    ):
                        nc.vector.tensor_scalar(
                            pu[:], pf[:], -2.0 / 255.0, 1.0, mul, add
                        )
                        nc.vector.tensor_mul(ps[:], pu[:], pu[:])
                        nc.vector.tensor_scalar(
                            pw[:], ps[:], _C5, _C3, mul, add
                        )
                        nc.vector.tensor_mul(pw2[:], pw[:], ps[:])
                        nc.vector.tensor_scalar_add(pw3[:], pw2[:], _C1)
                        nc.vector.tensor_mul(st[:, 0:F], pw3[:], pu[:])
                else:
                    it = pool.tile([P, F], u8, tag=f"in{t}")
                    # First load dispatched from the ACT sequencer (also
                    # HWDGE on TRN2): its runtime prolog ends ~2 us
                    # before Sync's, so the DMA stream starts that much
                    # earlier. Loads carry no sem waits, so unlike
                    # stores they cannot stall the ACT sequencer's
                    # activation stream.
                    (nc.scalar if t == 0 else nc.sync).dma_start(
                        out=it[:], in_=x_t
                    )
                    if t == 0:
                        # Dependency-free 1-element Sin: hoists the
                        # Sin-set table load off the tile-0-sem path.
                        nc.scalar.activation(
                            scratch.ap(), bias_ap,
                            mybir.ActivationFunctionType.Sin,
                            bias=bias_ap, scale=1.0,
                        )
                    # Three rotating fp16 sin buffers (u8 in-place is
                    # impossible): ACT(t) WARs only against DVE(t-3);
                    # two buffers measurably stalled the ACT sequencer
                    # ~0.3 us late in the stream.
                    st = pool.tile([P, F_MAX], f16, tag=f"sin{t % 3}")
                    nc.scalar.activation(
                        st[:, 0:F], it[:], mybir.ActivationFunctionType.Sin,
                        bias=bias_ap, scale=_SCALE,
                    )
                # Host pre-grouped each row as [A0|A1|A2|A3] (pixel k of
                # sample c at column k*C+c), so the sum of 4 is three
                # contiguous step-1 fp16 adds -> DVE 2x_1P packed mode.
                t1 = pool.tile([P, C], f16, tag="t1")
                t2 = pool.tile([P, C], f16, tag="t2")
                with nc.allow_low_precision(
                    "fp16 sum of 4 cos values; output gate is 2e-2 rel err"
                ):
                    nc.vector.tensor_add(t1[:], st[:, 0:C], st[:, C:2 * C])
                    nc.vector.tensor_add(
                        t2[:], st[:, 2 * C:3 * C], st[:, 3 * C:4 * C]
                    )
                    nc.vector.tensor_add(obufs[t][:], t1[:], t2[:])
                stores.append((y_t, obufs[t]))
                in_off += P * F
                out_off += P * C
            # All output DMAs after every input DMA in the Sync engine's
            # program order: the sequencer blocks on the first store's
            # wait only after all input DMAs are dispatched, and the
            # FIFO ring then drains inputs before outputs (inputs gate
            # compute; outputs are fire-and-forget).
            for y_t, ob in stores:
                nc.sync.dma_start(out=y_t, in_=ob[:])
    nc.finalize()
    return nc


_NC_CACHE = None


def _get_nc() -> bass.Bass:
    global _NC_CACHE
    if _NC_CACHE is None:
        _NC_CACHE = _build_nc()
    return _NC_CACHE


def _shard_inputs(x: np.ndarray) -> np.ndarray:
    """x: (B, 4) float32. Returns (N_CORES, L) uint8 in device layout:
    per core, per tile t, per partition p, the row is the tile's C_t
    samples grouped by pixel position [A0|A1|A2|A3]."""
    x8 = np.rint(x).astype(np.uint8).reshape(N_CORES, N_PER_CORE, N_PIX)
    xdev = np.empty((N_CORES, L), dtype=np.uint8)
    s0 = 0
    off = 0
    for F, C in zip(F_SCHED, C_SCHED):
        ns = P * C  # samples in this tile
        blk = (
            x8[:, s0:s0 + ns, :]
            .reshape(N_CORES, P, C, N_PIX)
            .transpose(0, 1, 3, 2)  # (cores, p, pix, c)
        )
        xdev[:, off:off + P * F] = blk.reshape(N_CORES, P * F)
        s0 += ns
        off += P * F
    return xdev


def _run(x: np.ndarray, **spmd_kwargs):
    """x: (B, 4) float32. Returns (full_output, BassKernelResults)."""
    xdev = _shard_inputs(x)
    in_maps = [{"x": xdev[i]} for i in range(N_CORES)]
    res = run_bass_kernel_spmd(_get_nc(), in_maps, list(range(N_CORES)), **spmd_kwargs)
    out = np.zeros((B, 3), dtype=np.float32)
    col = np.concatenate([r["y"] for r in res.results])  # (B,) fp16 sums
    out[:, 2] = col.astype(np.float32) * (1.0 / N_PIX)
    return out, res


def kernel(**inputs: np.ndarray) -> np.ndarray:
    x = np.ascontiguousarray(
        np.asarray(inputs["inputs"], dtype=np.float32)
    ).reshape(B, N_PIX)
    out, _ = _run(x)
    return out


# revision 18
# speedup vs baseline: 1.0704x; 1.0704x over previous
"""FRQI encoding kernel for Trainium2 (8 NeuronCores, data-parallel).

Closed form of the reference: for each sample b with 4 pixels x[b, 0:4],
  out[b] = [0.0, 0.0, mean_i cos(x[b, i] * pi / 255)]
The two address-qubit columns are input-independent and exactly zero
(mean over 4 pixel indices of (-1)^bit is 0 for both address bits), so
they are filled on the host; the device computes only the color column.

The ScalarE Sin stream (1 elem/cycle/lane @ 1.2 GHz -> ~15 us/core for
the 2.1M pixels) is the hard floor, so every other resource is sized to
never stall it, staying well inside the 2e-2 rel-err gate:
  - inputs are quantized to uint8 on the host (np.rint; the data is
    8-bit pixel intensities, quantization adds ~3.6e-3 rel err):
    2 MiB/core read. fp16 inputs (4 MiB) measurably starved the ACT
    stream in the early phase (effective supply ~4.5 us/MiB with
    dispatch+receipt overheads vs ACT demand ~3.6 us/MiB-equivalent).
  - the device emits the per-sample SUM of the 4 cos values in fp16
    (1 MiB/core); the host applies the 1/4 and writes the zero columns.

Device kernel (per core, 524288 samples = 2097152 uint8 pixels):
  - tiles of (128 partitions x F u8), contiguous DMA in; ladder
    schedule: tile F grows at the rate the measured DMA supply curve
    (sem_k ~ 8.9us + 0.55ns/elem) stays ahead of ACT consumption
    (0.833 ns/elem), so the Sin stream runs gap-free from first tile;
    small last tiles shrink the trailing adds+store.
  - ScalarE activation Sin(pi/2 - x*pi/255) == +cos(2*theta), u8 in /
    fp16 out into two alternating buffers (the HW Sin spline is only
    accurate on ~[-pi, pi]; the +pi/2 bias keeps arguments in
    (-pi/2, pi/2]).
  - a dependency-free 1-element Sin is issued right after the tile-0
    DMA dispatch so insert_act_table_loads places the Sin-set
    ACT_TABLE_LOAD (~1.3 us) before it, overlapping tile 0's DMA
    instead of serializing after its semaphore.
  - the host pre-groups each tile row as [pix0-blk|pix1-blk|pix2-blk|
    pix3-blk] so the sum-of-4 is three CONTIGUOUS fp16 tensor_adds on
    VectorE: step-1 16-bit operands hit the 2x_1P packed mode (stride-2
    pairwise adds would fall back to 1x and ~double DVE time).
  - contiguous fp16 DMA out of the (128, F/4) sum tile; all stores
    sit behind all loads in the Sync ring's FIFO so they never steal
    SDMA bandwidth from loads that gate compute.
"""

import math
import sys

for _p in ("/opt/trn_rl_repo",):
    if _p not in sys.path:
        sys.path.append(_p)

import numpy as np

# If the environment forces tracing (BASS_TRACE=1), run_bass_kernel_spmd
# imports antenv.axon_hooks, which this image lacks — stub it (only when
# absent) so the trace path degrades to "hook isn't registered" instead
# of crashing the kernel.
try:
    import antenv.axon_hooks  # noqa: F401
except ImportError:
    import types as _types

    _m = _types.ModuleType("antenv.axon_hooks")
    _m.get_axon_ntff_profile_hook = lambda: None
    _m.set_axon_ntff_profile_hook = lambda h: None
    sys.modules["antenv.axon_hooks"] = _m

import concourse.bass as bass
import concourse.mybir as mybir
from concourse import bacc
from concourse.bass_utils import run_bass_kernel_spmd
from concourse.tile import TileContext

N_CORES = 8
B = 4_194_304
N_PIX = 4
N_PER_CORE = B // N_CORES          # 524288 samples
P = 128                            # SBUF partitions
L = N_PER_CORE * N_PIX             # 2097152 u8 pixels per core
LO = N_PER_CORE                    # 524288 fp16 sums per core

# Per-tile free-dim sizes (u8 elems per partition; sum = L/P = 16384).
# Big tiles sit mid-stream: a late 4096-tile's three DVE adds (~1.9 us)
# would outlive the short trailing sins and push the last store out.
# Early tiles stay <=1024: completion semaphores arrive at a ~2 us
# cadence under load regardless of size, so fine granularity keeps the
# Sin stream fed during the supply ramp.
F_SCHED = [512, 1024, 2048, 4096, 4096, 1024, 2048, 1024, 512]
# Tile 5 is computed by a degree-5 odd polynomial on the (otherwise
# half-idle) VectorE instead of ScalarE Sin: cos(pi*x/255) =
# sin(pi/2*u) for u = 1 - 2x/255, evaluated as ((c5*s + c3)*s + c1)*u
# with s = u^2 (max err 1.1e-4, far below the u8 quantization noise).
# This shaves ~1.1 us off the critical ScalarE stream. Its load rides
# the scalar HWDGE ring right behind tile 0 (data by ~10.5 us without
# displacing any Sin-gating sync-ring load), and its DVE ops are
# emitted after tile 2's adds so they fill the early-supply stall
# windows where VectorE would otherwise idle.
POLY_T = 5
POLY_EMIT = 3  # poly DVE ops emitted just before this tile's adds
assert sum(F_SCHED) * P == L
C_SCHED = [f // N_PIX for f in F_SCHED]
F_MAX = max(F_SCHED)
_C1, _C3, _C5 = 1.5706268, -0.6432292, 0.0727102

# cos(z) = sin(pi/2 - z) for z = x*pi/255 = 2*theta: with scale=-pi/255
# and bias=+pi/2 the activation argument stays in (-pi/2, pi/2], the
# accurate domain of the HW Sin spline (it degrades badly beyond ~pi),
# and no sign fix-up is needed downstream.
_SCALE = -math.pi / 255.0
_BIAS = math.pi / 2.0


def _make_bacc() -> bacc.Bacc:
    """Construct Bacc without its init-time const-AP memsets and
    all-engine barrier. Nothing reads the four built-in const APs here
    (birverifier reports all four as "no reader"), and without the
    barrier each engine reaches its first kernel instruction as soon as
    its own runtime prolog finishes — the Activation engine then
    dispatches the first input DMA ~2 us before the Sync engine could.
    The patched methods are restored before any kernel instruction is
    traced."""
    sh = bass.BassSharedVectorInterface
    saved_memset = sh.memset
    saved_barrier = bass.Bass.all_engine_barrier
    sh.memset = lambda self, ap, constant: None
    bass.Bass.all_engine_barrier = lambda self, *a, **k: None
    try:
        return bacc.Bacc()
    finally:
        sh.memset = saved_memset
        bass.Bass.all_engine_barrier = saved_barrier


def _build_nc() -> bass.Bass:
    # Bacc (not raw Bass): its compile() pass generate_event_semaphores
    # splits multi-sem waits to satisfy the 1-wait-per-instruction HW limit.
    nc = _make_bacc()
    u8 = mybir.dt.uint8
    f16 = mybir.dt.float16
    f32 = mybir.dt.float32
    x = nc.dram_tensor("x", [L], u8, kind="ExternalInput")
    y = nc.dram_tensor("y", [LO], f16, kind="ExternalOutput")

    bias_t = nc.alloc_sbuf_tensor("bias_pi2", [P, 1], f32)
    scratch = nc.alloc_sbuf_tensor("act_warm", [P, 1], f16)
    # Persistent per-tile output buffers: stores never WAR with later
    # tiles' compute.
    obufs = [
        nc.alloc_sbuf_tensor(f"ob{t}", [P, c], f16)
        for t, c in enumerate(C_SCHED)
    ]
    bias_ap = bias_t.ap()

    in_offs = [0]
    out_offs = [0]
    for F in F_SCHED:
        in_offs.append(in_offs[-1] + P * F)
        out_offs.append(out_offs[-1] + P * (F // N_PIX))

    mul = mybir.AluOpType.mult
    add = mybir.AluOpType.add

    with TileContext(nc) as tc:
        # One slot per uniquely-tagged input tile: no slot reuse, so no
        # in-DMA ever carries a WAR wait and the Sync sequencer can
        # dispatch every input DMA up front.
        with tc.tile_pool(name="io", bufs=1) as pool:
            nc.gpsimd.memset(bias_ap, _BIAS)

            def x_ap(t):
                return x[in_offs[t]:in_offs[t + 1]].rearrange(
                    "(p f) -> p f", p=P
                )

            def grouped_adds(st, t):
                # Host pre-grouped each row as [A0|A1|A2|A3] (pixel k of
                # sample c at column k*C+c), so the sum of 4 is three
                # contiguous step-1 fp16 adds -> DVE 2x_1P packed mode.
                C = C_SCHED[t]
                t1 = pool.tile([P, C], f16, tag="t1")
                t2 = pool.tile([P, C], f16, tag="t2")
                with nc.allow_low_precision(
                    "fp16 sum of 4 cos values; gate is 2e-2 rel err"
                ):
                    nc.vector.tensor_add(t1[:], st[:, 0:C], st[:, C:2 * C])
                    nc.vector.tensor_add(
                        t2[:], st[:, 2 * C:3 * C], st[:, 3 * C:4 * C]
                    )
                    nc.vector.tensor_add(obufs[t][:], t1[:], t2[:])

            def poly_tile():
                # u8->fp32 conversion is free inside the first
                # tensor_scalar read (engines compute in fp32; a SWDGE
                # cast-during-DMA was tried instead and produced
                # intermittent partial-tile corruption).
                F = F_SCHED[POLY_T]
                pu = pool.tile([P, F], f16, tag="pu")
                ps = pool.tile([P, F], f16, tag="ps")
                pw = pool.tile([P, F], f16, tag="pw")
                pw2 = pool.tile([P, F], f16, tag="pw2")
                py = pool.tile([P, F], f16, tag="py")
                with nc.allow_low_precision(
                    "fp16 poly cosine; gate is 2e-2 rel err"
                ):
                    nc.vector.tensor_scalar(
                        pu[:], pf[:], -2.0 / 255.0, 1.0, mul, add
                    )
                    nc.vector.tensor_mul(ps[:], pu[:], pu[:])
                    nc.vector.tensor_scalar(pw[:], ps[:], _C5, _C3, mul, add)
                    nc.vector.tensor_mul(pw2[:], pw[:], ps[:])
                    # y = (pw2 + c1) * u in one fused op
                    nc.vector.scalar_tensor_tensor(
                        py[:], pw2[:], _C1, pu[:], add, mul
                    )
                grouped_adds(py, POLY_T)

            stores = []
            for t, F in enumerate(F_SCHED):
                C = F // N_PIX
                y_t = y[out_offs[t]:out_offs[t + 1]].rearrange(
                    "(p f) -> p f", p=P
                )
                stores.append((y_t, obufs[t]))
                if t == POLY_T:
                    continue  # DVE ops for it were emitted at POLY_EMIT
                it = pool.tile([P, F], u8, tag=f"in{t}")
                # First load dispatched from the ACT sequencer (also
                # HWDGE on TRN2): its runtime prolog ends ~2 us before
                # Sync's, so the DMA stream starts that much earlier.
                # Loads carry no sem waits, so unlike stores they cannot
                # stall the ACT sequencer's activation stream.
                (nc.scalar if t == 0 else nc.sync).dma_start(
                    out=it[:], in_=x_ap(t)
                )
                if t == 0:
                    # Poly tile's load rides the scalar ring second:
                    # lands ~10.5 us without displacing any Sin-gating
                    # sync load.
                    pf = pool.tile([P, F_SCHED[POLY_T]], u8, tag="pf")
                    nc.scalar.dma_start(out=pf[:], in_=x_ap(POLY_T))
                    # Dependency-free 1-element Sin: hoists the Sin-set
                    # table load off the tile-0-sem path.
                    nc.scalar.activation(
                        scratch.ap(), bias_ap,
                        mybir.ActivationFunctionType.Sin,
                        bias=bias_ap, scale=1.0,
                    )
                if t == POLY_EMIT:
                    poly_tile()
                # Three rotating fp16 sin buffers (u8 in-place is
                # impossible): ACT(t) WARs only against DVE reads three
                # sin-tiles back; two buffers measurably stalled the ACT
                # sequencer ~0.3 us late in the stream.
                st = pool.tile([P, F_MAX], f16, tag=f"sin{t % 3}")
                nc.scalar.activation(
                    st[:, 0:F], it[:], mybir.ActivationFunctionType.Sin,
                    bias=bias_ap, scale=_SCALE,
                )
                grouped_adds(st, t)
            # All output DMAs after every input DMA in the Sync engine's
            # program order: the sequencer blocks on the first store's
            # wait only after all input DMAs are dispatched, and the
            # FIFO ring then drains inputs before outputs (inputs gate
            # compute; outputs are fire-and-forget).
            for y_t, ob in stores:
                nc.sync.dma_start(out=y_t, in_=ob[:])
    nc.finalize()
    return nc


_NC_CACHE = None


def _get_nc() -> bass.Bass:
    global _NC_CACHE
    if _NC_CACHE is None:
        _NC_CACHE = _build_nc()
    return _NC_CACHE


def _shard_inputs(x: np.ndarray) -> np.ndarray:
    """x: (B, 4) float32. Returns (N_CORES, L) uint8 in device layout:
    per core, per tile t, per partition p, the row is the tile's C_t
    samples grouped by pixel position [A0|A1|A2|A3]."""
    x8 = np.rint(x).astype(np.uint8).reshape(N_CORES, N_PER_CORE, N_PIX)
    xdev = np.empty((N_CORES, L), dtype=np.uint8)
    s0 = 0
    off = 0
    for F, C in zip(F_SCHED, C_SCHED):
        ns = P * C  # samples in this tile
        blk = (
            x8[:, s0:s0 + ns, :]
            .reshape(N_CORES, P, C, N_PIX)
            .transpose(0, 1, 3, 2)  # (cores, p, pix, c)
        )
        xdev[:, off:off + P * F] = blk.reshape(N_CORES, P * F)
        s0 += ns
        off += P * F
    return xdev


def _run(x: np.ndarray, **spmd_kwargs):
    """x: (B, 4) float32. Returns (full_output, BassKernelResults)."""
    xdev = _shard_inputs(x)
    in_maps = [{"x": xdev[i]} for i in range(N_CORES)]
    res = run_bass_kernel_spmd(_get_nc(), in_maps, list(range(N_CORES)), **spmd_kwargs)
    out = np.zeros((B, 3), dtype=np.float32)
    col = np.concatenate([r["y"] for r in res.results])  # (B,) fp16 sums
    out[:, 2] = col.astype(np.float32) * (1.0 / N_PIX)
    return out, res


def kernel(**inputs: np.ndarray) -> np.ndarray:
    x = np.ascontiguousarray(
        np.asarray(inputs["inputs"], dtype=np.float32)
    ).reshape(B, N_PIX)
    out, _ = _run(x)
    return out
